# revision 1
# baseline (speedup 1.0000x reference)
"""Embedding lookup (nn_CustomEmbedding) on 8 Trainium2 NeuronCores.

reference: out[b, t, :] = weight.T[index[b, t], :]
  index:  [4096, 200] int32  (values in [0, 100000))
  weight: [128, 100000] f32
  out:    [4096, 200, 128] f32

Strategy (data-parallel batch shard, replicated table, 2-pass parity
dma_gather on two SWDGE queues):
  - Host: ext = [weight.T ; zero rows] -> [100130, D] contiguous 512B rows.
  - Shard the 819200 flat lookups across 8 cores (102400 each).
  - Gathers run as hardware-looped `dma_gather` (int16 indices). int16 spans
    only 65536 row-addresses, so the table is covered in TWO passes with a
    2-row (1KB) stride and a signed mid-window base:
      pass E: base row 65536, idx=(v>>1)-32768 addresses all EVEN rows
      pass O: base row 65537, same idx formula addresses all ODD rows
    Slots whose lookup has the other parity read one of 64 spread-out zero
    rows appended after the table; dst_E + dst_O (DVE add) merges the passes.
  - The two passes are issued on DIFFERENT SWDGE queues (queue_num 0/1,
    num_swdge_queues=2): their Q7 descriptor generation overlaps (~1.7x),
    which is what beats the one-desc-per-lookup indirect-DMA schedule.
  - Slot layout is chosen so every store is a fully regular 128-partition
    DMA with NCOLS*512B contiguous bytes per partition.

Measured (8 cores, NTFF): ~1.04ms, bit-exact. Pool desc-gen remains the
bottleneck: SWDGE generates data-dependent descriptors at ~8.6ns each
serial, ~4.7ns effective with two queues; 2 passes x 102432 descs/core.
(The indirect_dma_start alternative — one desc per lookup, no dummies —
measures 1.15ms and cannot use queue parallelism: walrus pins InstDMACopy
to queue 0 regardless of the BIR queue field.)
"""

import numpy as np

import concourse.bacc as bacc
import concourse.mybir as mybir
import concourse.tile as tile
from concourse.bass_utils import run_bass_kernel_spmd

V = 100000
D = 128
EXT = 100130  # V vocab rows + 130 zero rows (64-way spread dummies + slack)
N_CORES = 8
N_TOTAL = 4096 * 200  # 819200
N_CORE = N_TOTAL // N_CORES  # 102400
NI = 3200  # lookups per gather instruction (before pad)
NIP = NI + 16  # +16 trailing always-positive dummies (defeats per-lane
#                trailing-negative truncation in the gather ucode)
NCOLS = NI // 128  # 50 columns of gathered rows per partition
NG = N_CORE // NI  # 16 groups
ICOLS = NIP // 16  # 401 int16 index columns in the 16-partition stripe
DUMMY_BASE = 17232  # idx of first zero row under the (v>>1)-32768 mapping

_cached = {}


def _build():
    nc = bacc.Bacc(
        "TRN2",
        target_bir_lowering=False,
        debug=False,
        enable_asserts=False,
        num_devices=N_CORES,
        num_swdge_queues=2,
    )
    idxE_dram = nc.dram_tensor(
        "idxE", [128, NG * ICOLS], mybir.dt.int16, kind="ExternalInput"
    )
    idxO_dram = nc.dram_tensor(
        "idxO", [128, NG * ICOLS], mybir.dt.int16, kind="ExternalInput"
    )
    ext_dram = nc.dram_tensor("ext", [EXT, D], mybir.dt.float32, kind="ExternalInput")
    out_dram = nc.dram_tensor(
        "out", [N_CORE, D], mybir.dt.float32, kind="ExternalOutput"
    )

    # even/odd row views with a 2-row (256-element) stride, based mid-window
    # so signed int16 indices reach the whole table
    even_view = (
        ext_dram.ap()[65536 : 65536 + 34592]
        .rearrange("(a two) d -> a two d", two=2)[:, 0, :]
    )
    odd_view = (
        ext_dram.ap()[65537 : 65537 + 34592]
        .rearrange("(a two) d -> a two d", two=2)[:, 0, :]
    )

    # out viewed as [NG, 128, NCOLS*D]: group g, partition p holds rows
    # g*NI + p*NCOLS .. +NCOLS-1 -- contiguous NCOLS*D elements.
    out_r = out_dram.ap().rearrange("(g p c) d -> g p (c d)", p=128, c=NCOLS)

    with tile.TileContext(nc) as tc:
        with (
            tc.tile_pool(name="idxp", bufs=1) as idx_pool,
            tc.tile_pool(name="ge", bufs=3) as gpool_e,
            tc.tile_pool(name="go", bufs=3) as gpool_o,
        ):
            idxE_tile = idx_pool.tile([128, NG * ICOLS], mybir.dt.int16)
            idxO_tile = idx_pool.tile([128, NG * ICOLS], mybir.dt.int16)
            nc.sync.dma_start(idxE_tile[:], idxE_dram.ap())
            nc.sync.dma_start(idxO_tile[:], idxO_dram.ap())
            for g in range(NG):
                dstE = gpool_e.tile([128, (NCOLS + 1) * D], mybir.dt.float32)
                dstO = gpool_o.tile([128, (NCOLS + 1) * D], mybir.dt.float32)
                nc.gpsimd.dma_gather(
                    out_ap=dstE[:].rearrange("p (c d) -> p c d", d=D),
                    in_ap=even_view,
                    idxs_ap=idxE_tile[:, g * ICOLS : (g + 1) * ICOLS],
                    num_idxs=NIP,
                    num_idxs_reg=NIP,
                    elem_size=D,
                    elem_step=2 * D,
                    single_packet=False,
                    queue_num=0,
                )
                nc.gpsimd.dma_gather(
                    out_ap=dstO[:].rearrange("p (c d) -> p c d", d=D),
                    in_ap=odd_view,
                    idxs_ap=idxO_tile[:, g * ICOLS : (g + 1) * ICOLS],
                    num_idxs=NIP,
                    num_idxs_reg=NIP,
                    elem_size=D,
                    elem_step=2 * D,
                    single_packet=False,
                    queue_num=1,
                )
                nc.vector.tensor_add(
                    out=dstE[:, : NCOLS * D],
                    in0=dstE[:, : NCOLS * D],
                    in1=dstO[:, : NCOLS * D],
                )
                nc.sync.dma_start(out_r[g], dstE[:, : NCOLS * D])
    nc.compile()
    return nc


def _get_nc():
    if "nc" not in _cached:
        _cached["nc"] = _build()
    return _cached["nc"]


# slot i (gather list position) <-> within-group position t: the gather
# writes entry i to dst[i % 128, i // 128], and partition p must hold
# positions p*NCOLS .. +NCOLS-1, so i = (t % NCOLS)*128 + (t // NCOLS).
_T_OF_SLOT = np.arange(NI).reshape(128, NCOLS).T.ravel()  # slot i -> t
_DUMMY = (DUMMY_BASE + (np.arange(NIP) & 63)).astype(np.int16)  # per-slot zero row


def _arrange_pass(vals: np.ndarray, keep: np.ndarray) -> np.ndarray:
    """Build the [128, NG*ICOLS] int16 index tensor for one parity pass.

    vals: int16 [N_CORE] gather index per position ((v>>1) - 32768)
    keep: bool [N_CORE] whether this position belongs to this pass
    The [16, ICOLS] stripe (entry i at [i%16, i//16]) is replicated 8x down
    the partitions -- one copy per GpSimd Q7 core.
    """
    out = np.empty((128, NG * ICOLS), dtype=np.int16)
    for g in range(NG):
        v_g = vals[g * NI : (g + 1) * NI]
        k_g = keep[g * NI : (g + 1) * NI]
        slots = _DUMMY.copy()
        slots[:NI][...] = np.where(k_g[_T_OF_SLOT], v_g[_T_OF_SLOT], _DUMMY[:NI])
        stripe = slots.reshape(ICOLS, 16).T  # [16, ICOLS]
        out[:, g * ICOLS : (g + 1) * ICOLS] = np.tile(stripe, (8, 1))
    return out


def make_in_maps(index: np.ndarray, weight: np.ndarray):
    idx_flat = np.ascontiguousarray(index, dtype=np.int64).reshape(-1)
    table = weight.T.astype(np.float32, copy=False)
    ext = np.zeros((EXT, D), dtype=np.float32)
    ext[:V] = table

    in_maps = []
    for c in range(N_CORES):
        v = idx_flat[c * N_CORE : (c + 1) * N_CORE]
        base = ((v >> 1) - 32768).astype(np.int16)
        even = (v & 1) == 0
        in_maps.append(
            {
                "idxE": _arrange_pass(base, even),
                "idxO": _arrange_pass(base, ~even),
                "ext": ext,
            }
        )
    return in_maps


def kernel(index: np.ndarray, weight: np.ndarray) -> np.ndarray:
    in_maps = make_in_maps(index, weight)
    nc = _get_nc()
    res = run_bass_kernel_spmd(nc, in_maps, core_ids=list(range(N_CORES)))
    outs = [r["out"] for r in res.results]
    full = np.concatenate(outs, axis=0)  # [819200, 128]
    return full.reshape(index.shape[0], index.shape[1], D)



# revision 4
# speedup vs baseline: 1.3482x; 1.3482x over previous
"""Embedding lookup (nn_CustomEmbedding) on 8 Trainium2 NeuronCores.

reference: out[b, t, :] = weight.T[index[b, t], :]
  index:  [4096, 200] int32  (values in [0, 100000))
  weight: [128, 100000] f32
  out:    [4096, 200, 128] f32

Strategy (data-parallel batch shard, replicated table, 2-pass parity
dma_gather on two SWDGE queues):
  - Host: ext = [weight.T ; zero rows] -> [100130, D] contiguous 512B rows.
  - Shard the 819200 flat lookups across 8 cores (102400 each).
  - Gathers run as hardware-looped `dma_gather` (int16 indices). int16 spans
    only 65536 row-addresses, so the table is covered in TWO passes with a
    2-row (1KB) stride and a signed mid-window base:
      pass E: base row 65536, idx=(v>>1)-32768 addresses all EVEN rows
      pass O: base row 65537, same idx formula addresses all ODD rows
    Slots whose lookup has the other parity read one of 64 spread-out zero
    rows appended after the table; dst_E + dst_O (DVE add) merges the passes.
  - The two passes are issued on DIFFERENT SWDGE queues (queue_num 0/1,
    num_swdge_queues=2): their Q7 descriptor generation overlaps (~1.7x),
    which is what beats the one-desc-per-lookup indirect-DMA schedule.
  - Slot layout is chosen so every store is a fully regular 128-partition
    DMA with NCOLS*512B contiguous bytes per partition.

Measured (8 cores, NTFF): ~1.04ms, bit-exact. Pool desc-gen remains the
bottleneck: SWDGE generates data-dependent descriptors at ~8.6ns each
serial, ~4.7ns effective with two queues; 2 passes x 102432 descs/core.
(The indirect_dma_start alternative — one desc per lookup, no dummies —
measures 1.15ms and cannot use queue parallelism: walrus pins InstDMACopy
to queue 0 regardless of the BIR queue field.)
"""

import numpy as np

import concourse.bacc as bacc
import concourse.mybir as mybir
import concourse.tile as tile
from concourse.bass_utils import run_bass_kernel_spmd

V = 100000
D = 128
EXT = 100130  # V vocab rows + 130 zero rows (64-way spread dummies + slack)
N_CORES = 8
N_TOTAL = 4096 * 200  # 819200
N_CORE = N_TOTAL // N_CORES  # 102400
NI = 3200  # lookups per gather instruction (before pad)
NIP = NI + 16  # +16 trailing always-positive dummies (defeats per-lane
#                trailing-negative truncation in the gather ucode)
NCOLS = NI // 128  # 50 columns of gathered rows per partition
NG = N_CORE // NI  # 16 groups
ICOLS = NIP // 16  # 401 int16 index columns in the 16-partition stripe
DUMMY_BASE = 17232  # idx of first zero row under the (v>>1)-32768 mapping

_cached = {}


def _build():
    nc = bacc.Bacc(
        "TRN2",
        target_bir_lowering=False,
        debug=False,
        enable_asserts=False,
        num_devices=N_CORES,
        num_swdge_queues=4,
    )
    idxE_dram = nc.dram_tensor(
        "idxE", [128, NG * ICOLS], mybir.dt.int16, kind="ExternalInput"
    )
    idxO_dram = nc.dram_tensor(
        "idxO", [128, NG * ICOLS], mybir.dt.int16, kind="ExternalInput"
    )
    ext_dram = nc.dram_tensor("ext", [EXT, D], mybir.dt.float32, kind="ExternalInput")
    out_dram = nc.dram_tensor(
        "out", [N_CORE, D], mybir.dt.float32, kind="ExternalOutput"
    )

    # even/odd row views with a 2-row (256-element) stride, based mid-window
    # so signed int16 indices reach the whole table
    even_view = (
        ext_dram.ap()[65536 : 65536 + 34592]
        .rearrange("(a two) d -> a two d", two=2)[:, 0, :]
    )
    odd_view = (
        ext_dram.ap()[65537 : 65537 + 34592]
        .rearrange("(a two) d -> a two d", two=2)[:, 0, :]
    )

    # out viewed as [NG, 128, NCOLS*D]: group g, partition p holds rows
    # g*NI + p*NCOLS .. +NCOLS-1 -- contiguous NCOLS*D elements.
    out_r = out_dram.ap().rearrange("(g p c) d -> g p (c d)", p=128, c=NCOLS)

    with tile.TileContext(nc) as tc:
        with (
            tc.tile_pool(name="idxp", bufs=1) as idx_pool,
            tc.tile_pool(name="ge", bufs=3) as gpool_e,
            tc.tile_pool(name="go", bufs=3) as gpool_o,
        ):
            idxE_tile = idx_pool.tile([128, NG * ICOLS], mybir.dt.int16)
            idxO_tile = idx_pool.tile([128, NG * ICOLS], mybir.dt.int16)
            nc.sync.dma_start(idxE_tile[:], idxE_dram.ap())
            nc.sync.dma_start(idxO_tile[:], idxO_dram.ap())
            for g in range(NG):
                dstE = gpool_e.tile([128, (NCOLS + 1) * D], mybir.dt.float32)
                dstO = gpool_o.tile([128, (NCOLS + 1) * D], mybir.dt.float32)
                nc.gpsimd.dma_gather(
                    out_ap=dstE[:].rearrange("p (c d) -> p c d", d=D),
                    in_ap=even_view,
                    idxs_ap=idxE_tile[:, g * ICOLS : (g + 1) * ICOLS],
                    num_idxs=NIP,
                    num_idxs_reg=NIP,
                    elem_size=D,
                    elem_step=2 * D,
                    single_packet=False,
                    queue_num=(g % 2) * 2,
                )
                nc.gpsimd.dma_gather(
                    out_ap=dstO[:].rearrange("p (c d) -> p c d", d=D),
                    in_ap=odd_view,
                    idxs_ap=idxO_tile[:, g * ICOLS : (g + 1) * ICOLS],
                    num_idxs=NIP,
                    num_idxs_reg=NIP,
                    elem_size=D,
                    elem_step=2 * D,
                    single_packet=False,
                    queue_num=(g % 2) * 2 + 1,
                )
                nc.vector.tensor_add(
                    out=dstE[:, : NCOLS * D],
                    in0=dstE[:, : NCOLS * D],
                    in1=dstO[:, : NCOLS * D],
                )
                nc.sync.dma_start(out_r[g], dstE[:, : NCOLS * D])
    nc.compile()
    return nc


def _get_nc():
    if "nc" not in _cached:
        _cached["nc"] = _build()
    return _cached["nc"]


# slot i (gather list position) <-> within-group position t: the gather
# writes entry i to dst[i % 128, i // 128], and partition p must hold
# positions p*NCOLS .. +NCOLS-1, so i = (t % NCOLS)*128 + (t // NCOLS).
_T_OF_SLOT = np.arange(NI).reshape(128, NCOLS).T.ravel()  # slot i -> t
_DUMMY = (DUMMY_BASE + (np.arange(NIP) & 63)).astype(np.int16)  # per-slot zero row


def _arrange_pass(vals: np.ndarray, keep: np.ndarray) -> np.ndarray:
    """Build the [128, NG*ICOLS] int16 index tensor for one parity pass.

    vals: int16 [N_CORE] gather index per position ((v>>1) - 32768)
    keep: bool [N_CORE] whether this position belongs to this pass
    The [16, ICOLS] stripe (entry i at [i%16, i//16]) is replicated 8x down
    the partitions -- one copy per GpSimd Q7 core.
    """
    out = np.empty((128, NG * ICOLS), dtype=np.int16)
    for g in range(NG):
        v_g = vals[g * NI : (g + 1) * NI]
        k_g = keep[g * NI : (g + 1) * NI]
        slots = _DUMMY.copy()
        slots[:NI][...] = np.where(k_g[_T_OF_SLOT], v_g[_T_OF_SLOT], _DUMMY[:NI])
        stripe = slots.reshape(ICOLS, 16).T  # [16, ICOLS]
        out[:, g * ICOLS : (g + 1) * ICOLS] = np.tile(stripe, (8, 1))
    return out


def make_in_maps(index: np.ndarray, weight: np.ndarray):
    idx_flat = np.ascontiguousarray(index, dtype=np.int64).reshape(-1)
    table = weight.T.astype(np.float32, copy=False)
    ext = np.zeros((EXT, D), dtype=np.float32)
    ext[:V] = table

    in_maps = []
    for c in range(N_CORES):
        v = idx_flat[c * N_CORE : (c + 1) * N_CORE]
        base = ((v >> 1) - 32768).astype(np.int16)
        even = (v & 1) == 0
        in_maps.append(
            {
                "idxE": _arrange_pass(base, even),
                "idxO": _arrange_pass(base, ~even),
                "ext": ext,
            }
        )
    return in_maps


def kernel(index: np.ndarray, weight: np.ndarray) -> np.ndarray:
    in_maps = make_in_maps(index, weight)
    nc = _get_nc()
    res = run_bass_kernel_spmd(nc, in_maps, core_ids=list(range(N_CORES)))
    outs = [r["out"] for r in res.results]
    full = np.concatenate(outs, axis=0)  # [819200, 128]
    return full.reshape(index.shape[0], index.shape[1], D)



# revision 7
# speedup vs baseline: 1.3721x; 1.0177x over previous
"""Embedding lookup (nn_CustomEmbedding) on 8 Trainium2 NeuronCores.

reference: out[b, t, :] = weight.T[index[b, t], :]
  index:  [4096, 200] int32/int64  (values in [0, 100000))
  weight: [128, 100000] f32
  out:    [4096, 200, 128] f32

Strategy (data-parallel batch shard, replicated bf16 table, single-pass
PAIR dma_gather on 4 SWDGE queues + DVE select):
  - Host: table -> bf16, viewed as 50000 PAIRS of rows (1 pair = 2 rows
    = 512B). One descriptor per lookup fetches the pair containing the
    row: idx = (v>>1) - 32768 (signed int16 spans 65536 pairs = 131072
    rows >= 100000), elem_size/elem_step = 512B.
  - This is HALF the descriptors of the 2-pass parity scheme (no dummy
    zero-row fetches) and HALF the gather bytes (bf16): SWDGE desc-gen
    and DMA bus load both drop ~2x.
  - On-chip, the wanted row of each pair is chosen by v&1: DVE
    copy_predicated (mask broadcast along the 128-elem row) overwrites
    the low half with the high half where v is odd, then tensor_copy
    upconverts bf16 -> f32 for the store.
  - 4 SWDGE queues (ucode max), group g's gather on queue g%4.
  - rel-err from bf16 rounding <= 2^-9, far inside the 2e-2 gate.
"""

import numpy as np

import concourse.bacc as bacc
import concourse.mybir as mybir
import concourse.tile as tile
from concourse.bass_utils import run_bass_kernel_spmd

V = 100000
D = 128
NPAIR = V // 2  # 50000 pairs of table rows
N_CORES = 8
N_TOTAL = 4096 * 200  # 819200
N_CORE = N_TOTAL // N_CORES  # 102400
NI = 6400  # lookups per gather instruction (before pad)
NIP = NI + 16  # +16 trailing always-nonneg dummies (defeats per-lane
#                trailing-negative truncation in the gather ucode)
NCOLS = NI // 128  # 50 columns of gathered pairs per partition
NG = N_CORE // NI  # 16 groups
ICOLS = NIP // 16  # 401 int16 index columns in the 16-partition stripe
N_QUEUES = 4

_cached = {}


def _build():
    nc = bacc.Bacc(
        "TRN2",
        target_bir_lowering=False,
        debug=False,
        enable_asserts=False,
        num_devices=N_CORES,
        num_swdge_queues=N_QUEUES,
    )
    idx_dram = nc.dram_tensor(
        "idx", [128, NG * ICOLS], mybir.dt.int16, kind="ExternalInput"
    )
    msk_dram = nc.dram_tensor(
        "msk", [128, NG * NCOLS], mybir.dt.uint8, kind="ExternalInput"
    )
    ext_dram = nc.dram_tensor("ext", [V, D], mybir.dt.bfloat16, kind="ExternalInput")
    out_dram = nc.dram_tensor(
        "out", [N_CORE, D], mybir.dt.float32, kind="ExternalOutput"
    )

    # pair view: entry a = table rows [2a, 2a+1] (512B), based mid-window at
    # pair 32768 (row 65536) so signed int16 indices reach all 50000 pairs
    pair_view = (
        ext_dram.ap()[65536:V].rearrange("(a two) d -> a (two d)", two=2)
    )

    # out viewed as [NG, 128, NCOLS*D]: group g, partition p holds rows
    # g*NI + p*NCOLS .. +NCOLS-1 -- contiguous NCOLS*D elements.
    out_r = out_dram.ap().rearrange("(g p c) d -> g p (c d)", p=128, c=NCOLS)

    with tile.TileContext(nc) as tc:
        with (
            tc.tile_pool(name="idxp", bufs=1) as idx_pool,
            tc.tile_pool(name="gp", bufs=3) as gpool,
            tc.tile_pool(name="op", bufs=2) as opool,
        ):
            idx_tile = idx_pool.tile([128, NG * ICOLS], mybir.dt.int16)
            msk_tile = idx_pool.tile([128, NG * NCOLS], mybir.dt.uint8)
            nc.sync.dma_start(idx_tile[:], idx_dram.ap())
            nc.sync.dma_start(msk_tile[:], msk_dram.ap())
            for g in range(NG):
                dst = gpool.tile([128, (NCOLS + 1) * 2 * D], mybir.dt.bfloat16)
                nc.gpsimd.dma_gather(
                    out_ap=dst[:].rearrange("p (c d) -> p c d", d=2 * D),
                    in_ap=pair_view,
                    idxs_ap=idx_tile[:, g * ICOLS : (g + 1) * ICOLS],
                    num_idxs=NIP,
                    num_idxs_reg=NIP,
                    elem_size=2 * D,
                    elem_step=2 * D,
                    single_packet=False,
                    queue_num=g % N_QUEUES,
                )
                pairs = dst[:].rearrange("p (c d) -> p c d", d=2 * D)
                lo = pairs[:, :NCOLS, 0:D]
                hi = pairs[:, :NCOLS, D : 2 * D]
                mexp = msk_tile[:, g * NCOLS : (g + 1) * NCOLS].broadcast_to(
                    [128, NCOLS, D]
                )
                # keep hi where v was odd, else lo -- written in place over lo
                nc.vector.copy_predicated(lo, mexp, hi)
                out_t = opool.tile([128, NCOLS * D], mybir.dt.float32)
                nc.vector.tensor_copy(
                    out_t[:].rearrange("p (c d) -> p c d", d=D), lo
                )
                nc.sync.dma_start(out_r[g], out_t[:])
    nc.compile()
    return nc


def _get_nc():
    if "nc" not in _cached:
        _cached["nc"] = _build()
    return _cached["nc"]


# slot i (gather list position) <-> within-group position t: the gather
# writes entry i to dst[i % 128, i // 128], and partition p must hold
# positions p*NCOLS .. +NCOLS-1, so i = (t % NCOLS)*128 + (t // NCOLS).
_T_OF_SLOT = np.arange(NI).reshape(128, NCOLS).T.ravel()  # slot i -> t


def make_in_maps(index: np.ndarray, weight: np.ndarray):
    import ml_dtypes

    idx_flat = np.ascontiguousarray(index, dtype=np.int64).reshape(-1)
    ext = np.ascontiguousarray(weight.T).astype(ml_dtypes.bfloat16)

    in_maps = []
    for c in range(N_CORES):
        v = idx_flat[c * N_CORE : (c + 1) * N_CORE]
        pair_idx = ((v >> 1) - 32768).astype(np.int16)  # [N_CORE]
        odd = (v & 1).astype(np.uint8)  # [N_CORE]

        idx_arr = np.empty((128, NG * ICOLS), dtype=np.int16)
        msk_arr = np.empty((128, NG * NCOLS), dtype=np.uint8)
        for g in range(NG):
            p_g = pair_idx[g * NI : (g + 1) * NI]
            o_g = odd[g * NI : (g + 1) * NI]
            # gather slot order
            slots = np.zeros(NIP, dtype=np.int16)  # 16 trailing dummies = 0
            slots[:NI] = p_g[_T_OF_SLOT]
            # [16, ICOLS] stripe (entry i at [i%16, i//16]), replicated 8x
            stripe = slots.reshape(ICOLS, 16).T
            idx_arr[:, g * ICOLS : (g + 1) * ICOLS] = np.tile(stripe, (8, 1))
            # mask in dst layout: [p, c] = odd(position p*NCOLS+c)
            msk_arr[:, g * NCOLS : (g + 1) * NCOLS] = o_g.reshape(128, NCOLS)
        in_maps.append({"idx": idx_arr, "msk": msk_arr, "ext": ext})
    return in_maps


def kernel(index: np.ndarray, weight: np.ndarray) -> np.ndarray:
    in_maps = make_in_maps(index, weight)
    nc = _get_nc()
    res = run_bass_kernel_spmd(nc, in_maps, core_ids=list(range(N_CORES)))
    outs = [r["out"] for r in res.results]
    full = np.concatenate(outs, axis=0)  # [819200, 128]
    return full.reshape(index.shape[0], index.shape[1], D)


# revision 9
# speedup vs baseline: 1.8089x; 1.3183x over previous
"""Embedding lookup (nn_CustomEmbedding) on 8 Trainium2 NeuronCores.

reference: out[b, t, :] = weight.T[index[b, t], :]
  index:  [4096, 200] int32/int64  (values in [0, 100000))
  weight: [128, 100000] f32
  out:    [4096, 200, 128] f32

Strategy (data-parallel batch shard, replicated bf16 table, single-pass
PAIR dma_gather on 4 SWDGE queues + DVE select):
  - Host: table -> bf16, viewed as 50000 PAIRS of rows (1 pair = 2 rows
    = 512B). One descriptor per lookup fetches the pair containing the
    row: idx = (v>>1) - 32768 (signed int16 spans 65536 pairs = 131072
    rows >= 100000), elem_size/elem_step = 512B.
  - This is HALF the descriptors of the 2-pass parity scheme (no dummy
    zero-row fetches) and HALF the gather bytes (bf16): SWDGE desc-gen
    and DMA bus load both drop ~2x.
  - On-chip, the wanted row of each pair is chosen by v&1: DVE
    copy_predicated (mask broadcast along the 128-elem row) overwrites
    the low half with the high half where v is odd, then tensor_copy
    upconverts bf16 -> f32 for the store.
  - 4 SWDGE queues (ucode max), group g's gather on queue g%4.
  - rel-err from bf16 rounding <= 2^-9, far inside the 2e-2 gate.
"""

import numpy as np

import concourse.bacc as bacc
import concourse.mybir as mybir
import concourse.tile as tile
from concourse.bass_utils import run_bass_kernel_spmd

V = 100000
D = 128
NPAIR = V // 2  # 50000 pairs of table rows
N_CORES = 8
N_TOTAL = 4096 * 200  # 819200
N_CORE = N_TOTAL // N_CORES  # 102400
NI = 6400  # lookups per gather instruction (before pad)
NIP = NI + 16  # +16 trailing always-nonneg dummies (defeats per-lane
#                trailing-negative truncation in the gather ucode)
NCOLS = NI // 128  # 50 columns of gathered pairs per partition
NG = N_CORE // NI  # 16 groups
ICOLS = NIP // 16  # 401 int16 index columns in the 16-partition stripe
N_QUEUES = 4

_cached = {}


def _build():
    nc = bacc.Bacc(
        "TRN2",
        target_bir_lowering=False,
        debug=False,
        enable_asserts=False,
        num_devices=N_CORES,
        num_swdge_queues=N_QUEUES,
    )
    idx_dram = nc.dram_tensor(
        "idx", [128, NG * ICOLS], mybir.dt.int16, kind="ExternalInput"
    )
    msk_dram = nc.dram_tensor(
        "msk", [128, NG * NCOLS], mybir.dt.uint8, kind="ExternalInput"
    )
    ext_dram = nc.dram_tensor("ext", [V, D], mybir.dt.bfloat16, kind="ExternalInput")
    out_dram = nc.dram_tensor(
        "out", [N_CORE, D], mybir.dt.float32, kind="ExternalOutput"
    )

    # pair view: entry a = table rows [2a, 2a+1] (512B), based mid-window at
    # pair 32768 (row 65536) so signed int16 indices reach all 50000 pairs
    pair_view = (
        ext_dram.ap()[65536:V].rearrange("(a two) d -> a (two d)", two=2)
    )

    # out viewed as [NG, 128, NCOLS*D]: group g, partition p holds rows
    # g*NI + p*NCOLS .. +NCOLS-1 -- contiguous NCOLS*D elements.
    out_r = out_dram.ap().rearrange("(g p c) d -> g p (c d)", p=128, c=NCOLS)

    with tile.TileContext(nc) as tc:
        with (
            tc.tile_pool(name="idxp", bufs=1) as idx_pool,
            tc.tile_pool(name="gp", bufs=4) as gpool,
            tc.tile_pool(name="op", bufs=2) as opool,
        ):
            idx_tile = idx_pool.tile([128, NG * ICOLS], mybir.dt.int16)
            msk_tile = idx_pool.tile([128, NG * NCOLS], mybir.dt.uint8)
            nc.sync.dma_start(idx_tile[:], idx_dram.ap())
            nc.sync.dma_start(msk_tile[:], msk_dram.ap())
            for g in range(NG):
                dst = gpool.tile([128, (NCOLS + 1) * 2 * D], mybir.dt.bfloat16)
                nc.gpsimd.dma_gather(
                    out_ap=dst[:].rearrange("p (c d) -> p c d", d=2 * D),
                    in_ap=pair_view,
                    idxs_ap=idx_tile[:, g * ICOLS : (g + 1) * ICOLS],
                    num_idxs=NIP,
                    num_idxs_reg=NIP,
                    elem_size=2 * D,
                    elem_step=2 * D,
                    single_packet=False,
                    queue_num=g % N_QUEUES,
                )
                pairs = dst[:].rearrange("p (c d) -> p c d", d=2 * D)
                lo = pairs[:, :NCOLS, 0:D]
                hi = pairs[:, :NCOLS, D : 2 * D]
                mexp = msk_tile[:, g * NCOLS : (g + 1) * NCOLS].broadcast_to(
                    [128, NCOLS, D]
                )
                # keep hi where v was odd, else lo -- written in place over lo
                nc.vector.copy_predicated(lo, mexp, hi)
                out_t = opool.tile([128, NCOLS * D], mybir.dt.float32)
                # bf16 -> f32 upconvert on the (otherwise idle) Act engine
                nc.scalar.copy(out_t[:].rearrange("p (c d) -> p c d", d=D), lo)
                nc.sync.dma_start(out_r[g], out_t[:])
    nc.compile()
    return nc


def _get_nc():
    if "nc" not in _cached:
        _cached["nc"] = _build()
    return _cached["nc"]


# slot i (gather list position) <-> within-group position t: the gather
# writes entry i to dst[i % 128, i // 128], and partition p must hold
# positions p*NCOLS .. +NCOLS-1, so i = (t % NCOLS)*128 + (t // NCOLS).
_T_OF_SLOT = np.arange(NI).reshape(128, NCOLS).T.ravel()  # slot i -> t


def make_in_maps(index: np.ndarray, weight: np.ndarray):
    import ml_dtypes

    idx_flat = np.ascontiguousarray(index, dtype=np.int64).reshape(-1)
    ext = np.ascontiguousarray(weight.T).astype(ml_dtypes.bfloat16)

    in_maps = []
    for c in range(N_CORES):
        v = idx_flat[c * N_CORE : (c + 1) * N_CORE]
        pair_idx = ((v >> 1) - 32768).astype(np.int16)  # [N_CORE]
        odd = (v & 1).astype(np.uint8)  # [N_CORE]

        idx_arr = np.empty((128, NG * ICOLS), dtype=np.int16)
        msk_arr = np.empty((128, NG * NCOLS), dtype=np.uint8)
        for g in range(NG):
            p_g = pair_idx[g * NI : (g + 1) * NI]
            o_g = odd[g * NI : (g + 1) * NI]
            # gather slot order
            slots = np.zeros(NIP, dtype=np.int16)  # 16 trailing dummies = 0
            slots[:NI] = p_g[_T_OF_SLOT]
            # [16, ICOLS] stripe (entry i at [i%16, i//16]), replicated 8x
            stripe = slots.reshape(ICOLS, 16).T
            idx_arr[:, g * ICOLS : (g + 1) * ICOLS] = np.tile(stripe, (8, 1))
            # mask in dst layout: [p, c] = odd(position p*NCOLS+c)
            msk_arr[:, g * NCOLS : (g + 1) * NCOLS] = o_g.reshape(128, NCOLS)
        in_maps.append({"idx": idx_arr, "msk": msk_arr, "ext": ext})
    return in_maps


def kernel(index: np.ndarray, weight: np.ndarray) -> np.ndarray:
    in_maps = make_in_maps(index, weight)
    nc = _get_nc()
    res = run_bass_kernel_spmd(nc, in_maps, core_ids=list(range(N_CORES)))
    outs = [r["out"] for r in res.results]
    full = np.concatenate(outs, axis=0)  # [819200, 128]
    return full.reshape(index.shape[0], index.shape[1], D)


# revision 10
# speedup vs baseline: 2.4790x; 1.3704x over previous
"""Embedding lookup (nn_CustomEmbedding) on 8 Trainium2 NeuronCores.

reference: out[b, t, :] = weight.T[index[b, t], :]
  index:  [4096, 200] int32/int64  (values in [0, 100000))
  weight: [128, 100000] f32
  out:    [4096, 200, 128] f32

Strategy (data-parallel batch shard, replicated bf16 table, single-pass
PAIR dma_gather on 4 SWDGE queues, raw-bass manual-semaphore pipeline):
  - Host: table -> bf16, viewed as 50000 PAIRS of rows (1 pair = 2 rows
    = 512B). ONE descriptor per lookup fetches the pair containing the
    row: idx = (v>>1) - 32768 (signed int16 spans 65536 pairs = 131072
    rows >= 100000), elem_size = elem_step = 512B.
  - On-chip, the wanted half of each pair is chosen by v&1: DVE
    copy_predicated (uint8 mask broadcast along the 128-elem row)
    overwrites the low half with the high half where v is odd.
  - The device stores bf16; the HOST upconverts to f32 (halves store
    traffic and removes the cast from the device critical path).
  - 4 SWDGE queues: the gather ucode runs each instruction's desc-gen on
    the Q7 core pair selected by queue_num (cpu_id/2 == queue_num), so 4
    queues = 4 fully parallel desc-gen streams. The TileContext version
    serialized desc-gen behind DMA-completion bridge events on Pool;
    here all cross-engine deps are manual sems waited on the CONSUMER:
      Pool:  back-to-back dma_gather (queue g%4) -> dma_sem[g%8] += 16
      DVE :  wait dma_sem        -> copy_predicated -> pred_sem += 1
      SP  :  idx/msk loads up front; wait pred_sem -> store (HWDGE)
             -> store_sem += 16;  Pool reuses a dst tile after the
             store of the group NBUF back completed.
  - rel-err from bf16 rounding <= 2^-9, far inside the 2e-2 gate.
"""

from contextlib import ExitStack

import numpy as np

import concourse.bacc as bacc
import concourse.mybir as mybir
from concourse.bass_utils import run_bass_kernel_spmd
from concourse.library_config import mlp

V = 100000
D = 128
N_CORES = 8
N_TOTAL = 4096 * 200  # 819200
N_CORE = N_TOTAL // N_CORES  # 102400
NI = 3200  # lookups per gather instruction (before pad)
NIP = NI + 16  # +16 trailing nonnegative dummies (the gather ucode trims
#                trailing-negative idxs, which would leave garbage slots)
NCOLS = NI // 128  # 25 gathered pairs per partition per group
NG = N_CORE // NI  # 32 groups
ICOLS = NIP // 16  # 201 int16 index columns in the 16-partition stripe
NDST = NIP // 128 + 1  # 26 pair columns in the gather dst tile
N_QUEUES = 4
NBUF = 8  # gather dst tiles in flight
N_DMA_SEMS = 8

_cached = {}


def _build():
    nc = bacc.Bacc(
        "TRN2",
        target_bir_lowering=False,
        debug=False,
        enable_asserts=False,
        num_devices=N_CORES,
        num_swdge_queues=N_QUEUES,
    )
    idx_dram = nc.dram_tensor(
        "idx", [128, NG * ICOLS], mybir.dt.int16, kind="ExternalInput"
    )
    msk_dram = nc.dram_tensor(
        "msk", [128, NG * NCOLS], mybir.dt.uint8, kind="ExternalInput"
    )
    ext_dram = nc.dram_tensor("ext", [V, D], mybir.dt.bfloat16, kind="ExternalInput")
    out_dram = nc.dram_tensor(
        "out", [N_CORE, D], mybir.dt.bfloat16, kind="ExternalOutput"
    )

    # pair view: entry a = table rows [2a, 2a+1] (512B), based mid-window at
    # pair 32768 (row 65536) so signed int16 indices reach all 50000 pairs
    pair_view = ext_dram.ap()[65536:V].rearrange("(a two) d -> a (two d)", two=2)

    # out group g, partition p holds positions g*NI + p*NCOLS .. +NCOLS-1
    out_r = out_dram.ap().rearrange("(g p c) d -> g p c d", p=128, c=NCOLS)

    with (
        nc.Block() as block,
        nc.sbuf_tensor("idx_sb", [128, NG * ICOLS], mybir.dt.int16) as idx_sb,
        nc.sbuf_tensor("msk_sb", [128, NG * NCOLS], mybir.dt.uint8) as msk_sb,
        nc.sbuf_tensor(
            "dst_sb", [128, NBUF, NDST, 2 * D], mybir.dt.bfloat16
        ) as dst_sb,
        nc.semaphore("ld") as ld_sem,
        nc.semaphore("pred") as pred_sem,
        nc.semaphore("store") as store_sem,
        ExitStack() as stack,
    ):
        dma_sems = [
            stack.enter_context(nc.semaphore(f"dma{i}")) for i in range(N_DMA_SEMS)
        ]

        def lo_hi_mexp(g):
            b = g % NBUF
            lo = dst_sb[:, b, :NCOLS, 0:D]
            hi = dst_sb[:, b, :NCOLS, D : 2 * D]
            mexp = msk_sb[:, g * NCOLS : (g + 1) * NCOLS].broadcast_to(
                [128, NCOLS, D]
            )
            return lo, hi, mexp

        @block.sync
        def _(sp):
            # msk first (ld>=16), then idx slice g (ld>=16*(g+2))
            sp.dma_start(msk_sb[:], msk_dram.ap()).then_inc(ld_sem, 16)
            for g in range(NG):
                sp.dma_start(
                    idx_sb[:, g * ICOLS : (g + 1) * ICOLS],
                    idx_dram.ap()[:, g * ICOLS : (g + 1) * ICOLS],
                ).then_inc(ld_sem, 16)
            for g in range(NG):
                sp.wait_ge(pred_sem, g + 1)
                lo, _, _ = lo_hi_mexp(g)
                sp.dma_start(out_r[g], lo).then_inc(store_sem, 16)
            sp.wait_ge(store_sem, 16 * NG)

        @block.gpsimd
        def _(gp):
            gp.load_library(mlp)
            for g in range(NG):
                if g >= NBUF:
                    gp.wait_ge(store_sem, 16 * (g - NBUF + 1))
                gp.wait_ge(ld_sem, 16 * (g + 2))
                b = g % NBUF
                gp.dma_gather(
                    out_ap=dst_sb[:, b],
                    in_ap=pair_view,
                    idxs_ap=idx_sb[:, g * ICOLS : (g + 1) * ICOLS],
                    num_idxs=NIP,
                    num_idxs_reg=NIP,
                    elem_size=2 * D,
                    elem_step=2 * D,
                    single_packet=False,
                    queue_num=g % N_QUEUES,
                ).then_inc(dma_sems[g % N_DMA_SEMS], 16)
            for k in range(N_DMA_SEMS):
                gp.wait_ge(dma_sems[k], 16 * ((NG - 1 - k) // N_DMA_SEMS + 1))

        @block.vector
        def _(dve):
            dve.wait_ge(ld_sem, 16)  # msk resident
            for g in range(NG):
                dve.wait_ge(dma_sems[g % N_DMA_SEMS], 16 * (g // N_DMA_SEMS + 1))
                lo, hi, mexp = lo_hi_mexp(g)
                dve.copy_predicated(lo, mexp, hi).then_inc(pred_sem, 1)

    nc.compile()
    return nc


def _get_nc():
    if "nc" not in _cached:
        _cached["nc"] = _build()
    return _cached["nc"]


# slot i (gather list position) <-> within-group position t: the gather
# writes entry i to dst[i % 128, i // 128], and partition p must hold
# positions p*NCOLS .. +NCOLS-1, so i = (t % NCOLS)*128 + (t // NCOLS).
_T_OF_SLOT = np.arange(NI).reshape(128, NCOLS).T.ravel()  # slot i -> t


def make_in_maps(index: np.ndarray, weight: np.ndarray):
    import ml_dtypes

    idx_flat = np.ascontiguousarray(index, dtype=np.int64).reshape(-1)
    ext = np.ascontiguousarray(weight.T).astype(ml_dtypes.bfloat16)

    in_maps = []
    for c in range(N_CORES):
        v = idx_flat[c * N_CORE : (c + 1) * N_CORE]
        pair_idx = ((v >> 1) - 32768).astype(np.int16)  # [N_CORE]
        odd = (v & 1).astype(np.uint8)  # [N_CORE]

        idx_arr = np.empty((128, NG * ICOLS), dtype=np.int16)
        msk_arr = np.empty((128, NG * NCOLS), dtype=np.uint8)
        for g in range(NG):
            p_g = pair_idx[g * NI : (g + 1) * NI]
            o_g = odd[g * NI : (g + 1) * NI]
            slots = np.zeros(NIP, dtype=np.int16)  # 16 trailing dummies = 0
            slots[:NI] = p_g[_T_OF_SLOT]
            # [16, ICOLS] stripe (entry i at [i%16, i//16]), replicated 8x
            # down the partitions -- one copy per Q7 core pair
            stripe = slots.reshape(ICOLS, 16).T
            idx_arr[:, g * ICOLS : (g + 1) * ICOLS] = np.tile(stripe, (8, 1))
            # mask in dst layout: [p, c] = v&1 of position p*NCOLS+c
            msk_arr[:, g * NCOLS : (g + 1) * NCOLS] = o_g.reshape(128, NCOLS)
        in_maps.append({"idx": idx_arr, "msk": msk_arr, "ext": ext})
    return in_maps


def kernel(index: np.ndarray, weight: np.ndarray) -> np.ndarray:
    in_maps = make_in_maps(index, weight)
    nc = _get_nc()
    res = run_bass_kernel_spmd(nc, in_maps, core_ids=list(range(N_CORES)))
    outs = [np.asarray(r["out"]).astype(np.float32) for r in res.results]
    full = np.concatenate(outs, axis=0)  # [819200, 128]
    return full.reshape(index.shape[0], index.shape[1], D)


# revision 15
# speedup vs baseline: 3.0063x; 1.2127x over previous
"""Embedding lookup (nn_CustomEmbedding) on 8 Trainium2 NeuronCores.

reference: out[b, t, :] = weight.T[index[b, t], :]
  index:  [4096, 200] int32/int64  (values in [0, 100000))
  weight: [128, 100000] f32
  out:    [4096, 200, 128] f32

Strategy (data-parallel batch shard, replicated bf16 table, single-pass
PAIR dma_gather on 4 SWDGE queues, raw-bass manual-semaphore pipeline):
  - Host: table -> bf16, viewed as 50000 PAIRS of rows (1 pair = 2 rows
    = 512B). ONE descriptor per lookup fetches the pair containing the
    row: idx = (v>>1) - 32768 (signed int16 spans 65536 pairs = 131072
    rows >= 100000), elem_size = elem_step = 512B.
  - On-chip, the wanted half of each pair is chosen by v&1: DVE
    copy_predicated (uint8 mask broadcast along the 128-elem row)
    overwrites the low half with the high half where v is odd.
  - The device stores bf16; the HOST upconverts to f32 (halves store
    traffic and removes the cast from the device critical path).
  - 4 SWDGE queues: the gather ucode runs each instruction's desc-gen on
    the Q7 core pair selected by queue_num (cpu_id/2 == queue_num), so 4
    queues = 4 fully parallel desc-gen streams. The TileContext version
    serialized desc-gen behind DMA-completion bridge events on Pool;
    here all cross-engine deps are manual sems waited on the CONSUMER:
      Pool:  back-to-back dma_gather (queue g%4) -> dma_sem[g%8] += 16
      DVE :  wait dma_sem        -> copy_predicated -> pred_sem += 1
      Act :  wait pred_sem -> copy lo into a CONTIGUOUS bf16 tile
             (a strided store source would shatter the HWDGE store into
             256B descriptors) -> act_sem += 1
      SP  :  idx/msk loads up front; wait act_sem -> contiguous store
             (HWDGE) -> store_sem += 16;  Pool reuses a dst tile after
             the Act copy of the group NBUF back (act_sem).
  - rel-err from bf16 rounding <= 2^-9, far inside the 2e-2 gate.
"""

from contextlib import ExitStack

import numpy as np

import concourse.bacc as bacc
import concourse.mybir as mybir
from concourse.bass_utils import run_bass_kernel_spmd
from concourse.library_config import mlp

V = 100000
D = 128
N_CORES = 8
N_TOTAL = 4096 * 200  # 819200
N_CORE = N_TOTAL // N_CORES  # 102400
NI = 3200  # lookups per gather instruction (before pad)
NIP = NI + 16  # +16 trailing nonnegative dummies (the gather ucode trims
#                trailing-negative idxs, which would leave garbage slots)
NCOLS = NI // 128  # 25 gathered pairs per partition per group
NG = N_CORE // NI  # 32 groups
ICOLS = NIP // 16  # 201 int16 index columns in the 16-partition stripe
NDST = NIP // 128 + 1  # 26 pair columns in the gather dst tile
N_QUEUES = 4
NBUF = 8  # gather dst tiles in flight
NOUT = 4  # contiguous store-staging tiles
N_DMA_SEMS = 8

_cached = {}


def _build():
    nc = bacc.Bacc(
        "TRN2",
        target_bir_lowering=False,
        debug=False,
        enable_asserts=False,
        num_devices=N_CORES,
        num_swdge_queues=N_QUEUES,
    )
    idx_dram = nc.dram_tensor(
        "idx", [128, NG * ICOLS], mybir.dt.int16, kind="ExternalInput"
    )
    msk_dram = nc.dram_tensor(
        "msk", [128, NG * NCOLS], mybir.dt.uint8, kind="ExternalInput"
    )
    ext_dram = nc.dram_tensor("ext", [V, D], mybir.dt.bfloat16, kind="ExternalInput")
    out_dram = nc.dram_tensor(
        "out", [N_CORE, D], mybir.dt.bfloat16, kind="ExternalOutput"
    )

    # pair view: entry a = table rows [2a, 2a+1] (512B), based mid-window at
    # pair 32768 (row 65536) so signed int16 indices reach all 50000 pairs
    pair_view = ext_dram.ap()[65536:V].rearrange("(a two) d -> a (two d)", two=2)

    # out group g, partition p holds positions g*NI + p*NCOLS .. +NCOLS-1
    out_r = out_dram.ap().rearrange("(g p c) d -> g p (c d)", p=128, c=NCOLS)

    with (
        nc.Block() as block,
        nc.sbuf_tensor("idx_sb", [128, NG * ICOLS], mybir.dt.int16) as idx_sb,
        nc.sbuf_tensor("msk_sb", [128, NG * NCOLS], mybir.dt.uint8) as msk_sb,
        nc.sbuf_tensor(
            "dst_sb", [128, NBUF, NDST, 2 * D], mybir.dt.bfloat16
        ) as dst_sb,
        nc.sbuf_tensor(
            "cont_sb", [128, NOUT, NCOLS * D], mybir.dt.bfloat16
        ) as cont_sb,
        nc.semaphore("ld") as ld_sem,
        nc.semaphore("pred") as pred_sem,
        nc.semaphore("act") as act_sem,
        nc.semaphore("store") as store_sem,
        ExitStack() as stack,
    ):
        dma_sems = [
            stack.enter_context(nc.semaphore(f"dma{i}")) for i in range(N_DMA_SEMS)
        ]

        def lo_hi_mexp(g):
            b = g % NBUF
            lo = dst_sb[:, b, :NCOLS, 0:D]
            hi = dst_sb[:, b, :NCOLS, D : 2 * D]
            mexp = msk_sb[:, g * NCOLS : (g + 1) * NCOLS].broadcast_to(
                [128, NCOLS, D]
            )
            return lo, hi, mexp

        @block.sync
        def _(sp):
            # msk first (ld>=16), then idx slice g (ld>=16*(g+2))
            sp.dma_start(msk_sb[:], msk_dram.ap()).then_inc(ld_sem, 16)
            for g in range(NG):
                sp.dma_start(
                    idx_sb[:, g * ICOLS : (g + 1) * ICOLS],
                    idx_dram.ap()[:, g * ICOLS : (g + 1) * ICOLS],
                ).then_inc(ld_sem, 16)
            for g in range(NG):
                sp.wait_ge(act_sem, g + 1)
                sp.dma_start(out_r[g], cont_sb[:, g % NOUT]).then_inc(
                    store_sem, 16
                )
            sp.wait_ge(store_sem, 16 * NG)

        @block.gpsimd
        def _(gp):
            gp.load_library(mlp)
            for g in range(NG):
                if g >= NBUF:
                    gp.wait_ge(act_sem, g - NBUF + 1)
                gp.wait_ge(ld_sem, 16 * (g + 2))
                b = g % NBUF
                gp.dma_gather(
                    out_ap=dst_sb[:, b],
                    in_ap=pair_view,
                    idxs_ap=idx_sb[:, g * ICOLS : (g + 1) * ICOLS],
                    num_idxs=NIP,
                    num_idxs_reg=NIP,
                    elem_size=2 * D,
                    elem_step=2 * D,
                    single_packet=False,
                    queue_num=g % N_QUEUES,
                ).then_inc(dma_sems[g % N_DMA_SEMS], 16)
            for k in range(N_DMA_SEMS):
                gp.wait_ge(dma_sems[k], 16 * ((NG - 1 - k) // N_DMA_SEMS + 1))

        @block.vector
        def _(dve):
            dve.wait_ge(ld_sem, 16)  # msk resident
            for g in range(NG):
                dve.wait_ge(dma_sems[g % N_DMA_SEMS], 16 * (g // N_DMA_SEMS + 1))
                lo, hi, mexp = lo_hi_mexp(g)
                dve.copy_predicated(lo, mexp, hi).then_inc(pred_sem, 1)

        @block.scalar
        def _(act):
            for g in range(NG):
                act.wait_ge(pred_sem, g + 1)
                if g >= NOUT:
                    act.wait_ge(store_sem, 16 * (g - NOUT + 1))
                lo, _, _ = lo_hi_mexp(g)
                act.copy(
                    cont_sb[:, g % NOUT].rearrange("p (c d) -> p c d", d=D), lo
                ).then_inc(act_sem, 1)

    nc.compile()
    return nc


def _get_nc():
    if "nc" not in _cached:
        _cached["nc"] = _build()
    return _cached["nc"]


# slot i (gather list position) <-> within-group position t: the gather
# writes entry i to dst[i % 128, i // 128], and partition p must hold
# positions p*NCOLS .. +NCOLS-1, so i = (t % NCOLS)*128 + (t // NCOLS).
_T_OF_SLOT = np.arange(NI).reshape(128, NCOLS).T.ravel()  # slot i -> t


def make_in_maps(index: np.ndarray, weight: np.ndarray):
    import ml_dtypes

    idx_flat = np.ascontiguousarray(index, dtype=np.int64).reshape(-1)
    ext = np.ascontiguousarray(weight.T).astype(ml_dtypes.bfloat16)

    in_maps = []
    for c in range(N_CORES):
        v = idx_flat[c * N_CORE : (c + 1) * N_CORE]
        pair_idx = ((v >> 1) - 32768).astype(np.int16)  # [N_CORE]
        odd = (v & 1).astype(np.uint8)  # [N_CORE]

        idx_arr = np.empty((128, NG * ICOLS), dtype=np.int16)
        msk_arr = np.empty((128, NG * NCOLS), dtype=np.uint8)
        for g in range(NG):
            p_g = pair_idx[g * NI : (g + 1) * NI]
            o_g = odd[g * NI : (g + 1) * NI]
            slots = np.zeros(NIP, dtype=np.int16)  # 16 trailing dummies = 0
            slots[:NI] = p_g[_T_OF_SLOT]
            # [16, ICOLS] stripe (entry i at [i%16, i//16]), replicated 8x
            # down the partitions -- one copy per Q7 core pair
            stripe = slots.reshape(ICOLS, 16).T
            idx_arr[:, g * ICOLS : (g + 1) * ICOLS] = np.tile(stripe, (8, 1))
            # mask in dst layout: [p, c] = v&1 of position p*NCOLS+c
            msk_arr[:, g * NCOLS : (g + 1) * NCOLS] = o_g.reshape(128, NCOLS)
        in_maps.append({"idx": idx_arr, "msk": msk_arr, "ext": ext})
    return in_maps


def kernel(index: np.ndarray, weight: np.ndarray) -> np.ndarray:
    in_maps = make_in_maps(index, weight)
    nc = _get_nc()
    res = run_bass_kernel_spmd(nc, in_maps, core_ids=list(range(N_CORES)))
    outs = [np.asarray(r["out"]).astype(np.float32) for r in res.results]
    full = np.concatenate(outs, axis=0)  # [819200, 128]
    return full.reshape(index.shape[0], index.shape[1], D)


# revision 19
# speedup vs baseline: 3.0453x; 1.0130x over previous
"""Embedding lookup (nn_CustomEmbedding) on 8 Trainium2 NeuronCores.

reference: out[b, t, :] = weight.T[index[b, t], :]
  index:  [4096, 200] int32/int64  (values in [0, 100000))
  weight: [128, 100000] f32
  out:    [4096, 200, 128] f32

Strategy (data-parallel batch shard, replicated bf16 table, single-pass
PAIR dma_gather on 4 SWDGE queues, raw-bass manual-semaphore pipeline):
  - Host: table -> bf16, viewed as 50000 PAIRS of rows (1 pair = 2 rows
    = 512B). ONE descriptor per lookup fetches the pair containing the
    row: idx = (v>>1) - 32768 (signed int16 spans 65536 pairs = 131072
    rows >= 100000), elem_size = elem_step = 512B.
  - On-chip, the wanted half of each pair is chosen by v&1: DVE
    copy_predicated (uint8 mask broadcast along the 128-elem row)
    overwrites the low half with the high half where v is odd.
  - The device stores bf16; the HOST upconverts to f32 (halves store
    traffic and removes the cast from the device critical path).
  - 4 SWDGE queues: the gather ucode runs each instruction's desc-gen on
    the Q7 core pair selected by queue_num (cpu_id/2 == queue_num), so 4
    queues = 4 fully parallel desc-gen streams. The TileContext version
    serialized desc-gen behind DMA-completion bridge events on Pool;
    here all cross-engine deps are manual sems waited on the CONSUMER:
      Pool:  back-to-back dma_gather (queue g%4) -> dma_sem[g%8] += 16
      DVE :  wait dma_sem        -> copy_predicated -> pred_sem += 1
      Act :  wait pred_sem -> copy lo into a CONTIGUOUS bf16 tile
             (a strided store source would shatter the HWDGE store into
             256B descriptors) -> act_sem += 1
      SP  :  idx/msk loads up front; wait act_sem -> contiguous store
             (HWDGE) -> store_sem += 16;  Pool reuses a dst tile after
             the Act copy of the group NBUF back (act_sem).
  - rel-err from bf16 rounding <= 2^-9, far inside the 2e-2 gate.
"""

from contextlib import ExitStack

import numpy as np

import concourse.bacc as bacc
import concourse.mybir as mybir
from concourse.bass_utils import run_bass_kernel_spmd
from concourse.library_config import mlp

V = 100000
D = 128
N_CORES = 8
N_TOTAL = 4096 * 200  # 819200
N_CORE = N_TOTAL // N_CORES  # 102400
# group sizes tapered: small groups at both ends prime the pipeline
# faster (first DVE/store starts sooner) and shrink the post-desc-gen
# DMA drain tail; +16 trailing nonnegative dummies per group (the gather
# ucode trims trailing-negative idxs, which would leave garbage slots)
SIZES = [1664] * 4 + [3200] * 28 + [1536] * 4  # multiples of 128, sum N_CORE
assert sum(SIZES) == N_CORE
NG = len(SIZES)
OFFS = [sum(SIZES[:g]) for g in range(NG)]  # position offset per group
NCOLS_G = [ni // 128 for ni in SIZES]  # gathered pairs per partition
ICOLS_G = [(ni + 16) // 16 for ni in SIZES]  # idx stripe columns
IOFF = [sum(ICOLS_G[:g]) for g in range(NG + 1)]
MOFF = [sum(NCOLS_G[:g]) for g in range(NG + 1)]
NDST = (max(SIZES) + 16) // 128 + 1  # 26 pair columns in the dst tile
N_QUEUES = 4
NBUF = 8  # gather dst tiles in flight
NOUT = 4  # contiguous store-staging tiles
N_DMA_SEMS = 8

_cached = {}


def _build():
    nc = bacc.Bacc(
        "TRN2",
        target_bir_lowering=False,
        debug=False,
        enable_asserts=False,
        num_devices=N_CORES,
        num_swdge_queues=N_QUEUES,
    )
    idx_dram = nc.dram_tensor(
        "idx", [128, IOFF[NG]], mybir.dt.int16, kind="ExternalInput"
    )
    msk_dram = nc.dram_tensor(
        "msk", [128, MOFF[NG]], mybir.dt.uint8, kind="ExternalInput"
    )
    ext_dram = nc.dram_tensor("ext", [V, D], mybir.dt.bfloat16, kind="ExternalInput")
    out_dram = nc.dram_tensor(
        "out", [N_CORE, D], mybir.dt.bfloat16, kind="ExternalOutput"
    )

    # pair view: entry a = table rows [2a, 2a+1] (512B), based mid-window at
    # pair 32768 (row 65536) so signed int16 indices reach all 50000 pairs
    pair_view = ext_dram.ap()[65536:V].rearrange("(a two) d -> a (two d)", two=2)

    # out group g, partition p holds positions OFFS[g] + p*NCOLS_G[g] ...
    def out_view(g):
        return (
            out_dram.ap()[OFFS[g] : OFFS[g] + SIZES[g]]
            .rearrange("(p c) d -> p (c d)", p=128, c=NCOLS_G[g])
        )

    with (
        nc.Block() as block,
        nc.sbuf_tensor("idx_sb", [128, IOFF[NG]], mybir.dt.int16) as idx_sb,
        nc.sbuf_tensor("msk_sb", [128, MOFF[NG]], mybir.dt.uint8) as msk_sb,
        nc.sbuf_tensor(
            "dst_sb", [128, NBUF, NDST, 2 * D], mybir.dt.bfloat16
        ) as dst_sb,
        nc.sbuf_tensor(
            "cont_sb", [128, NOUT, (max(SIZES) // 128) * D], mybir.dt.bfloat16
        ) as cont_sb,
        nc.semaphore("ld") as ld_sem,
        nc.semaphore("pred") as pred_sem,
        nc.semaphore("act") as act_sem,
        nc.semaphore("store") as store_sem,
        ExitStack() as stack,
    ):
        dma_sems = [
            stack.enter_context(nc.semaphore(f"dma{i}")) for i in range(N_DMA_SEMS)
        ]

        def lo_hi_mexp(g):
            b = g % NBUF
            nc_g = NCOLS_G[g]
            lo = dst_sb[:, b, :nc_g, 0:D]
            hi = dst_sb[:, b, :nc_g, D : 2 * D]
            mexp = msk_sb[:, MOFF[g] : MOFF[g + 1]].broadcast_to(
                [128, nc_g, D]
            )
            return lo, hi, mexp

        @block.sync
        def _(sp):
            # idx slice 0 first so gather 0 can start ASAP, then msk,
            # then the remaining idx slices: slice g ready at ld>=16*(g+2)
            sp.dma_start(
                idx_sb[:, IOFF[0] : IOFF[1]], idx_dram.ap()[:, IOFF[0] : IOFF[1]]
            ).then_inc(ld_sem, 16)
            sp.dma_start(msk_sb[:], msk_dram.ap()).then_inc(ld_sem, 16)
            for g in range(1, NG):
                sp.dma_start(
                    idx_sb[:, IOFF[g] : IOFF[g + 1]],
                    idx_dram.ap()[:, IOFF[g] : IOFF[g + 1]],
                ).then_inc(ld_sem, 16)
            for g in range(NG):
                sp.wait_ge(act_sem, g + 1)
                sp.dma_start(
                    out_view(g), cont_sb[:, g % NOUT, : SIZES[g] * D // 128]
                ).then_inc(store_sem, 16)
            sp.wait_ge(store_sem, 16 * NG)

        @block.gpsimd
        def _(gp):
            gp.load_library(mlp)
            for g in range(NG):
                if g >= NBUF:
                    gp.wait_ge(act_sem, g - NBUF + 1)
                gp.wait_ge(ld_sem, 16 * (1 if g == 0 else g + 2))
                b = g % NBUF
                nip_g = SIZES[g] + 16
                gp.dma_gather(
                    out_ap=dst_sb[:, b, : nip_g // 128 + 1],
                    in_ap=pair_view,
                    idxs_ap=idx_sb[:, IOFF[g] : IOFF[g + 1]],
                    num_idxs=nip_g,
                    num_idxs_reg=nip_g,
                    elem_size=2 * D,
                    elem_step=2 * D,
                    single_packet=False,
                    queue_num=g % N_QUEUES,
                ).then_inc(dma_sems[g % N_DMA_SEMS], 16)
            for k in range(N_DMA_SEMS):
                gp.wait_ge(dma_sems[k], 16 * ((NG - 1 - k) // N_DMA_SEMS + 1))

        @block.vector
        def _(dve):
            dve.wait_ge(ld_sem, 32)  # msk resident (2nd load)
            for g in range(NG):
                dve.wait_ge(dma_sems[g % N_DMA_SEMS], 16 * (g // N_DMA_SEMS + 1))
                lo, hi, mexp = lo_hi_mexp(g)
                dve.copy_predicated(lo, mexp, hi).then_inc(pred_sem, 1)

        @block.scalar
        def _(act):
            for g in range(NG):
                act.wait_ge(pred_sem, g + 1)
                if g >= NOUT:
                    act.wait_ge(store_sem, 16 * (g - NOUT + 1))
                lo, _, _ = lo_hi_mexp(g)
                act.copy(
                    cont_sb[:, g % NOUT, : SIZES[g] * D // 128].rearrange(
                        "p (c d) -> p c d", d=D
                    ),
                    lo,
                ).then_inc(act_sem, 1)

    nc.compile()
    return nc


def _get_nc():
    if "nc" not in _cached:
        _cached["nc"] = _build()
    return _cached["nc"]


# slot i (gather list position) <-> within-group position t: the gather
# writes entry i to dst[i % 128, i // 128], and partition p must hold
# positions p*ncols .. +ncols-1, so i = (t % ncols)*128 + (t // ncols).
_T_OF_SLOT = {
    ni: np.arange(ni).reshape(128, ni // 128).T.ravel() for ni in set(SIZES)
}


def make_in_maps(index: np.ndarray, weight: np.ndarray):
    import ml_dtypes

    idx_flat = np.ascontiguousarray(index, dtype=np.int64).reshape(-1)
    ext = np.ascontiguousarray(weight.T).astype(ml_dtypes.bfloat16)

    in_maps = []
    for c in range(N_CORES):
        v = idx_flat[c * N_CORE : (c + 1) * N_CORE]
        pair_idx = ((v >> 1) - 32768).astype(np.int16)  # [N_CORE]
        odd = (v & 1).astype(np.uint8)  # [N_CORE]

        idx_arr = np.empty((128, IOFF[NG]), dtype=np.int16)
        msk_arr = np.empty((128, MOFF[NG]), dtype=np.uint8)
        for g in range(NG):
            ni = SIZES[g]
            p_g = pair_idx[OFFS[g] : OFFS[g] + ni]
            o_g = odd[OFFS[g] : OFFS[g] + ni]
            slots = np.zeros(ni + 16, dtype=np.int16)  # trailing dummies = 0
            slots[:ni] = p_g[_T_OF_SLOT[ni]]
            # [16, icols] stripe (entry i at [i%16, i//16]), replicated 8x
            # down the partitions -- one copy per Q7 core pair
            stripe = slots.reshape(ICOLS_G[g], 16).T
            idx_arr[:, IOFF[g] : IOFF[g + 1]] = np.tile(stripe, (8, 1))
            # mask in dst layout: [p, c] = v&1 of position p*ncols+c
            msk_arr[:, MOFF[g] : MOFF[g + 1]] = o_g.reshape(128, ni // 128)
        in_maps.append({"idx": idx_arr, "msk": msk_arr, "ext": ext})
    return in_maps


def kernel(index: np.ndarray, weight: np.ndarray) -> np.ndarray:
    in_maps = make_in_maps(index, weight)
    nc = _get_nc()
    res = run_bass_kernel_spmd(nc, in_maps, core_ids=list(range(N_CORES)))
    outs = [np.asarray(r["out"]).astype(np.float32) for r in res.results]
    full = np.concatenate(outs, axis=0)  # [819200, 128]
    return full.reshape(index.shape[0], index.shape[1], D)
